# revision 13
# baseline (speedup 1.0000x reference)
"""Bahdanau-style cosine attention kernel for Trainium2 (8 NeuronCores).

reference math (fp32):
    q = squeeze(query)              # [H]
    dots = keys @ q                 # [S]
    cos = dots / (|q| * |keys_i|)   # [S]
    context = sum_i cos_i * keys_i  # [H]

Sharding: keys split along S across 8 cores (4096 rows each); host
normalizes q by |q| and casts everything to fp16 (rel err ~2e-4 vs the
2e-2 gate) so each core streams 8 MiB of keys instead of 16 MiB.

Engine model (measured): free-dim accumulate passes exist only on DVE
(scalar_tensor_tensor, ~1.3 us/tile — no 2x uop) and ACT (Square+accum,
~1.1 us/tile); with 64 passes (32 dots + 32 squares) those two engines
are the wall. So the upper half of the rows is uploaded twice: row
layout (for squares + context) AND transposed (kthi[p, b, r] =
K[2048+r, 128b+p]), letting the idle PE compute their dots as matmuls
with the query as stationary weights: per 128-row window, 8 matmuls of
N=128 accumulate over the 8 column blocks into a [1,128] PSUM row,
which DVE/ACT drain to SBUF and an idle-DMA reshape scatters into
[128, tile] layout for the cos chain. Extra DMA: +4 MiB (12.6 total,
still under the engine makespan). DVE keeps the 16 low dot passes plus
6 squares; ACT gets 26 squares; both land ~34 us.

Context matmuls accumulate in two PSUM bank pairs: pair A (everything
except the last hi group) stops and ships its half of the output early;
pair B covers the tail group so the final drain is short.
"""

import os
import sys

import numpy as np

for _p in ("/opt/trn_rl_repo",):
    if os.path.isdir(_p) and _p not in sys.path:
        sys.path.append(_p)

P = 128          # SBUF partitions
H = 1024         # feature dim
NB = H // P      # column blocks = 8
S_FULL = 32768   # full sequence
N_CORES = 8
S = S_FULL // N_CORES   # rows per core = 4096
T = S // P              # row-tiles per core = 32
T_LO = 16               # tiles whose dots run on DVE (rows 0..2047)
T_HI = T - T_LO         # tiles whose dots run on the PE via kthi
R_HI = T_HI * P         # 2048 transposed rows

# klo chunks (tiles 0..15) double as cos groups
LO_CHUNKS = [(0, 2), (2, 4), (4, 8), (8, 12), (12, 16)]
# khi row-layout chunks (tiles 16..31), 4 tiles each = 4 hi groups
HI_CHUNKS = [(16, 20), (20, 24), (24, 28), (28, 32)]
# kthi (transposed) chunks, in 4-window units (1 window = 128 rows)
KTHI_CHUNKS = [(0, 4), (4, 8), (8, 12), (12, 16)]
# lo tiles whose square runs on DVE (accum in SBUF); must cover whole
# groups so each cos chain reads a single nrm2 tensor
DVE_SQ_GROUPS = frozenset({1, 3})   # tiles 2,3 and 8..11
PE_WARMUP_MMS = 6

_NC_CACHE = {}


def _build_nc():
    import concourse.bacc as bacc
    import concourse.tile as tile
    from concourse import mybir

    f32 = mybir.dt.float32
    f16 = mybir.dt.float16
    AF = mybir.ActivationFunctionType
    OP = mybir.AluOpType
    nc = bacc.Bacc("TRN2", target_bir_lowering=False, debug=False)

    keys_d = nc.dram_tensor("keys", [S, H], f16, kind="ExternalInput").ap()
    kthi_d = nc.dram_tensor("kthi", [P, NB * R_HI], f16,
                            kind="ExternalInput").ap()
    qb_d = nc.dram_tensor("qb", [P, H], f16, kind="ExternalInput").ap()
    qt_d = nc.dram_tensor("qt", [P, NB], f16, kind="ExternalInput").ap()
    ctx_d = nc.dram_tensor("ctx", [2, H], f32, kind="ExternalOutput").ap()

    with tile.TileContext(nc) as tc:
        with (
            tc.tile_pool(name="main", bufs=1) as pool,
            tc.tile_pool(name="psum", bufs=1, space="PSUM") as pp,
        ):
            qb = pool.tile([P, H], f16, name="qb_sb")
            qt = pool.tile([P, NB], f16, name="qt_sb")
            nc.sync.dma_start(qb[:], qb_d[:])
            nc.sync.dma_start(qt[:], qt_d[:])

            # keys[t*P + p, c] -> sbuf[p, t, c]: tile t = rows 128t..128t+127,
            # matching the kthi window/drain/reshape row order exactly
            keys_r = keys_d.rearrange("(t p) c -> p t c", p=P)
            kthi_r = kthi_d.rearrange("p (b r) -> p b r", b=NB)
            kthi = pool.tile([P, NB, R_HI], f16, name="kthi_sb")

            kcs = {}

            def load_keys(t0, t1, name):
                kc = pool.tile([P, (t1 - t0) * H], f16, name=name, tag=name)
                nc.sync.dma_start(kc[:], keys_r[:, t0:t1, :])
                for i in range(t0, t1):
                    kcs[i] = (kc, i - t0)

            def ktile(t):
                kc, i = kcs[t]
                return kc[:, i * H : (i + 1) * H]

            # DMA issue order: klo/kthi interleaved early (DVE + PE both
            # hungry), khi row layout last (ACT reaches it ~22us in)
            load_keys(0, 2, "klo0")
            load_keys(2, 4, "klo1")
            nc.sync.dma_start(kthi[:, :, 0:512], kthi_r[:, :, 0:512])
            load_keys(4, 8, "klo2")
            nc.sync.dma_start(kthi[:, :, 512:1024], kthi_r[:, :, 512:1024])
            load_keys(8, 12, "klo3")
            load_keys(16, 20, "khi0")
            nc.sync.dma_start(kthi[:, :, 1024:1536], kthi_r[:, :, 1024:1536])
            load_keys(12, 16, "klo4")
            load_keys(20, 24, "khi1")
            nc.sync.dma_start(kthi[:, :, 1536:2048], kthi_r[:, :, 1536:2048])
            load_keys(24, 28, "khi2")
            load_keys(28, 32, "khi3")

            dots = pool.tile([P, T_LO], f32, name="dots")
            dh = pool.tile([P, T_HI], f32, name="dh")
            dhfl = pool.tile([1, R_HI], f32, name="dhfl")
            nrm2p = pp.tile([P, T], f32, name="nrm2p")
            nrm2s = pool.tile([P, T_LO], f32, name="nrm2s")
            knrm = pool.tile([P, T], f32, name="knrm")
            rkn = pool.tile([P, T], f32, name="rkn")
            cosb = pool.tile([P, T], f16, name="cosb")
            dvescr = pool.tile([P, H], f16, name="dvescr")
            actscr = pp.tile([P, H], f32, name="actscr")
            psD = pp.tile([1, P], f32, name="psD")
            psA0 = pp.tile([1, 512], f32, name="psA0")
            psA1 = pp.tile([1, 512], f32, name="psA1")
            psB0 = pp.tile([1, 512], f32, name="psB0")
            psB1 = pp.tile([1, 512], f32, name="psB1")

            for _ in range(PE_WARMUP_MMS):
                nc.tensor.matmul(psB0[:], qb[:, 0:1], qb[:, 0:512],
                                 start=True, stop=True)

            firstA = {"v": True}
            firstB = {"v": True}

            def emit_lo_elem(gi, g0, g1):
                dve_sq = gi in DVE_SQ_GROUPS
                for t in range(g0, g1):
                    nc.vector.scalar_tensor_tensor(
                        out=dvescr[:], in0=ktile(t), scalar=1.0, in1=qb[:],
                        op0=OP.mult, op1=OP.mult,
                        accum_out=dots[:, t : t + 1],
                    )
                    if dve_sq:
                        nc.vector.scalar_tensor_tensor(
                            out=dvescr[:], in0=ktile(t), scalar=1.0,
                            in1=ktile(t), op0=OP.mult, op1=OP.mult,
                            accum_out=nrm2s[:, t : t + 1],
                        )
                    else:
                        nc.scalar.activation(
                            actscr[:], ktile(t), AF.Square,
                            accum_out=nrm2p[:, t : t + 1],
                        )

            def emit_cos(cols, dsrc, nsrc):
                with tc.high_priority(offset=40):
                    nc.scalar.activation(knrm[:, cols], nsrc, AF.Sqrt)
                    nc.vector.reciprocal(rkn[:, cols], knrm[:, cols])
                    nc.vector.tensor_mul(cosb[:, cols], dsrc, rkn[:, cols])

            def emit_ctx(g0, g1, pair, stop_last=False):
                p0, p1, first = pair
                for t in range(g0, g1):
                    kt = ktile(t)
                    st = first["v"]
                    first["v"] = False
                    stop = stop_last and t == g1 - 1
                    nc.tensor.matmul(p0[:], cosb[:, t : t + 1],
                                     kt[:, 0:512], start=st, stop=stop)
                    nc.tensor.matmul(p1[:], cosb[:, t : t + 1],
                                     kt[:, 512:1024], start=st, stop=stop)

            pairA = (psA0, psA1, firstA)
            pairB = (psB0, psB1, firstB)

            def emit_hi_window(w):
                # dots for hi rows 128w..128w+127 on the PE; drain to
                # dhfl on DVE/ACT (alternating), reshape to dh via DMA
                for b in range(NB):
                    nc.tensor.matmul(
                        psD[:], qt[:, b : b + 1],
                        kthi[:, b, 128 * w : 128 * (w + 1)],
                        start=(b == 0), stop=(b == NB - 1),
                    )
                dst = dhfl[:, 128 * w : 128 * (w + 1)]
                with tc.high_priority(offset=40):
                    if w % 2 == 0:
                        nc.vector.tensor_copy(dst, psD[:])
                    else:
                        nc.scalar.copy(dst, psD[:])
                nc.sync.dma_start(dh[:, w : w + 1], dst)

            def emit_lo_group(gi):
                g0, g1 = LO_CHUNKS[gi]
                emit_lo_elem(gi, g0, g1)
                nsrc = (nrm2s if gi in DVE_SQ_GROUPS else nrm2p)
                emit_cos(slice(g0, g1), dots[:, g0:g1], nsrc[:, g0:g1])
                emit_ctx(g0, g1, pairA)

            # interleave lo groups and hi windows so the PE FIFO never
            # parks a late-cos ctx matmul in front of ready window work
            emit_lo_group(0)
            emit_hi_window(0)
            emit_hi_window(1)
            emit_lo_group(1)
            emit_hi_window(2)
            emit_hi_window(3)
            emit_lo_group(2)
            for w in range(4, 8):
                emit_hi_window(w)
            emit_lo_group(3)
            for w in range(8, 12):
                emit_hi_window(w)
            emit_lo_group(4)
            for w in range(12, 16):
                emit_hi_window(w)

            # hi groups: squares on ACT from the row layout, cos from
            # (dh, nrm2p), ctx from the row layout
            for hg, (g0, g1) in enumerate(HI_CHUNKS):
                for t in range(g0, g1):
                    nc.scalar.activation(
                        actscr[:], ktile(t), AF.Square,
                        accum_out=nrm2p[:, t : t + 1],
                    )
                emit_cos(slice(g0, g1), dh[:, g0 - T_LO : g1 - T_LO],
                         nrm2p[:, g0:g1])
                last = hg == len(HI_CHUNKS) - 1
                if not last:
                    emit_ctx(g0, g1, pairA,
                             stop_last=(hg == len(HI_CHUNKS) - 2))
                else:
                    ctxA = pool.tile([1, H], f32, name="ctxA")
                    nc.scalar.copy(ctxA[:, 0:512], psA0[:])
                    nc.vector.tensor_copy(ctxA[:, 512:1024], psA1[:])
                    nc.sync.dma_start(ctx_d[0:1, :], ctxA[:])
                    emit_ctx(g0, g1, pairB, stop_last=True)

            ctxB = pool.tile([1, H], f32, name="ctxB")
            nc.scalar.copy(ctxB[:, 0:512], psB0[:])
            nc.vector.tensor_copy(ctxB[:, 512:1024], psB1[:])
            nc.sync.dma_start(ctx_d[1:2, :], ctxB[:])

    nc.compile()
    return nc


def _get_nc():
    if "nc" not in _NC_CACHE:
        _NC_CACHE["nc"] = _build_nc()
    return _NC_CACHE["nc"]


def prepare_in_maps(query: np.ndarray, keys: np.ndarray) -> list[dict]:
    query = np.asarray(query, dtype=np.float32)
    keys = np.asarray(keys, dtype=np.float32)
    assert query.shape == (1, H) and keys.shape == (S_FULL, H)

    q = query.reshape(H).astype(np.float64)
    qn = (q / np.linalg.norm(q)).astype(np.float16)
    qb = np.ascontiguousarray(np.broadcast_to(qn[None, :], (P, H)))
    qt = np.ascontiguousarray(qn.reshape(NB, P).T)  # qt[p, b] = qn[128b+p]

    keys16 = keys.astype(np.float16)
    shards = keys16.reshape(N_CORES, S, H)
    in_maps = []
    for i in range(N_CORES):
        sh = np.ascontiguousarray(shards[i])
        hi = sh[T_LO * P :]                      # [R_HI, H]
        # kthi[p, b, r] = hi[r, 128b + p]
        kthi = np.ascontiguousarray(
            hi.T.reshape(NB, P, R_HI).transpose(1, 0, 2)
        ).reshape(P, NB * R_HI)
        in_maps.append({"keys": sh, "kthi": kthi, "qb": qb, "qt": qt})
    return in_maps


def combine_results(results: list[dict]) -> np.ndarray:
    partials = np.stack([results[i]["ctx"] for i in range(N_CORES)])
    out = partials.astype(np.float64).sum(axis=(0, 1)).astype(np.float32)
    return out[None, :]


def kernel(query: np.ndarray, keys: np.ndarray) -> np.ndarray:
    from concourse.bass_utils import run_bass_kernel_spmd

    in_maps = prepare_in_maps(query, keys)
    nc = _get_nc()
    res = run_bass_kernel_spmd(nc, in_maps, list(range(N_CORES)))
    return combine_results(res.results)


# revision 16
# speedup vs baseline: 1.0170x; 1.0170x over previous
"""Bahdanau-style cosine attention kernel for Trainium2 (8 NeuronCores).

reference math (fp32):
    q = squeeze(query)              # [H]
    dots = keys @ q                 # [S]
    cos = dots / (|q| * |keys_i|)   # [S]
    context = sum_i cos_i * keys_i  # [H]

Sharding: keys split along S across 8 cores (4096 rows each); host
normalizes q by |q| and casts everything to fp16 (rel err ~2e-4 vs the
2e-2 gate) so each core streams 8 MiB of keys instead of 16 MiB.

Engine model (measured): free-dim accumulate passes exist only on DVE
(scalar_tensor_tensor ~1.3 us/tile, no 2x uop) and ACT (Square+accum
~1.1 us/tile); 64 passes on two engines is the wall. So rows 2048-4095
are uploaded twice: row layout (squares + context) AND transposed
(kthi), letting the idle PE compute their 16 dot passes as matmuls
with the query as stationary weights: per 512-row window, 8 matmuls of
N=512 accumulate the 8 column blocks into a [1,512] PSUM row; DVE/ACT
drain it to SBUF and an idle-DMA scatter reshapes it into [128, 4]
columns for the cos chain. DVE keeps 16 low dot passes + 6 squares,
ACT 26 squares; both land ~34 us next to a ~31 us DMA stream.

Both tensors are host-packed so every DMA chunk is per-partition
contiguous (128 large descriptors per chunk): HWDGE descriptor
generation on the sync queue is the issue-path bottleneck otherwise.

Context matmuls accumulate in two PSUM bank pairs: pair A (all but the
last hi group) stops and ships its half of the output early; pair B
covers the tail group so the final drain is short.
"""

import os
import sys

import numpy as np

for _p in ("/opt/trn_rl_repo",):
    if os.path.isdir(_p) and _p not in sys.path:
        sys.path.append(_p)

P = 128          # SBUF partitions
H = 1024         # feature dim
NB = H // P      # column blocks = 8
S_FULL = 32768   # full sequence
N_CORES = 8
S = S_FULL // N_CORES   # rows per core = 4096
T = S // P              # row-tiles per core = 32
T_LO = 16               # tiles whose dots run on DVE (rows 0..2047)
T_HI = T - T_LO         # tiles whose dots run on the PE via kthi
R_HI = T_HI * P         # 2048 transposed rows
W_ROWS = 512            # kthi window rows (= 1 PSUM bank of f32)
N_WIN = R_HI // W_ROWS  # 4 windows, 1:1 with the hi groups

# klo chunks (tiles 0..15) double as cos groups
LO_CHUNKS = [(0, 2), (2, 4), (4, 8), (8, 12), (12, 16)]
# khi row-layout chunks (tiles 16..31) = hi groups = kthi windows
HI_CHUNKS = [(16, 20), (20, 24), (24, 28), (28, 32)]
# lo groups whose square runs on DVE (accum in SBUF)
DVE_SQ_GROUPS = frozenset({1, 3})   # tiles 2,3 and 8..11
PE_WARMUP_MMS = 6

_NC_CACHE = {}


def _build_nc():
    import concourse.bacc as bacc
    import concourse.tile as tile
    from concourse import mybir

    f32 = mybir.dt.float32
    f16 = mybir.dt.float16
    AF = mybir.ActivationFunctionType
    OP = mybir.AluOpType
    nc = bacc.Bacc("TRN2", target_bir_lowering=False, debug=False)

    # keys packed [p, t*H + c] = keys[128t + p, c]; kthi packed
    # [p, w*NB*W_ROWS + b*W_ROWS + r] = keys[2048 + 512w + r, 128b + p]
    keys_d = nc.dram_tensor("keys", [P, T * H], f16, kind="ExternalInput").ap()
    kthi_d = nc.dram_tensor("kthi", [P, NB * R_HI], f16,
                            kind="ExternalInput").ap()
    qb_d = nc.dram_tensor("qb", [P, H], f16, kind="ExternalInput").ap()
    qt_d = nc.dram_tensor("qt", [P, NB], f16, kind="ExternalInput").ap()
    ctx_d = nc.dram_tensor("ctx", [2, H], f32, kind="ExternalOutput").ap()

    with tile.TileContext(nc) as tc:
        with (
            tc.tile_pool(name="main", bufs=1) as pool,
            tc.tile_pool(name="psum", bufs=1, space="PSUM") as pp,
        ):
            qb = pool.tile([P, H], f16, name="qb_sb")
            qt = pool.tile([P, NB], f16, name="qt_sb")
            nc.sync.dma_start(qb[:], qb_d[:])
            nc.sync.dma_start(qt[:], qt_d[:])

            kthi = pool.tile([P, NB * R_HI], f16, name="kthi_sb")
            kcs = {}

            def load_keys(t0, t1, name):
                kc = pool.tile([P, (t1 - t0) * H], f16, name=name, tag=name)
                nc.sync.dma_start(kc[:], keys_d[:, t0 * H : t1 * H])
                for i in range(t0, t1):
                    kcs[i] = (kc, i - t0)

            def load_kthi(w):
                wb = NB * W_ROWS
                nc.sync.dma_start(kthi[:, w * wb : (w + 1) * wb],
                                  kthi_d[:, w * wb : (w + 1) * wb])

            def ktile(t):
                kc, i = kcs[t]
                return kc[:, i * H : (i + 1) * H]

            def kthi_rhs(w, b):
                base = w * NB * W_ROWS + b * W_ROWS
                return kthi[:, base : base + W_ROWS]

            # DMA issue order: klo + kthi early (DVE and PE both hungry),
            # khi row layout later (ACT reaches it ~22us in)
            load_keys(0, 2, "klo0")
            load_keys(2, 4, "klo1")
            load_kthi(0)
            load_keys(4, 8, "klo2")
            load_kthi(1)
            load_keys(8, 12, "klo3")
            load_keys(16, 20, "khi0")
            load_kthi(2)
            load_keys(12, 16, "klo4")
            load_keys(20, 24, "khi1")
            load_kthi(3)
            load_keys(24, 28, "khi2")
            load_keys(28, 32, "khi3")

            dots = pool.tile([P, T_LO], f32, name="dots")
            dh = pool.tile([P, T_HI], f32, name="dh")
            dhfl = pool.tile([1, R_HI], f32, name="dhfl")
            nrm2p = pp.tile([P, T], f32, name="nrm2p")
            nrm2s = pool.tile([P, T_LO], f32, name="nrm2s")
            knrm = pool.tile([P, T], f32, name="knrm")
            rkn = pool.tile([P, T], f32, name="rkn")
            cosb = pool.tile([P, T], f16, name="cosb")
            dvescr = pool.tile([P, H], f16, name="dvescr")
            actscr = pp.tile([P, H], f32, name="actscr")
            psD = pp.tile([1, W_ROWS], f32, name="psD")
            psA0 = pp.tile([1, 512], f32, name="psA0")
            psA1 = pp.tile([1, 512], f32, name="psA1")
            psB0 = pp.tile([1, 512], f32, name="psB0")
            psB1 = pp.tile([1, 512], f32, name="psB1")

            for _ in range(PE_WARMUP_MMS):
                nc.tensor.matmul(psB0[:], qb[:, 0:1], qb[:, 0:512],
                                 start=True, stop=True)

            firstA = {"v": True}
            firstB = {"v": True}

            def emit_lo_elem(gi, g0, g1):
                dve_sq = gi in DVE_SQ_GROUPS
                for t in range(g0, g1):
                    nc.vector.scalar_tensor_tensor(
                        out=dvescr[:], in0=ktile(t), scalar=1.0, in1=qb[:],
                        op0=OP.mult, op1=OP.mult,
                        accum_out=dots[:, t : t + 1],
                    )
                    if dve_sq:
                        nc.vector.scalar_tensor_tensor(
                            out=dvescr[:], in0=ktile(t), scalar=1.0,
                            in1=ktile(t), op0=OP.mult, op1=OP.mult,
                            accum_out=nrm2s[:, t : t + 1],
                        )
                    else:
                        nc.scalar.activation(
                            actscr[:], ktile(t), AF.Square,
                            accum_out=nrm2p[:, t : t + 1],
                        )

            def emit_cos(cols, dsrc, nsrc):
                with tc.high_priority(offset=40):
                    nc.scalar.activation(knrm[:, cols], nsrc, AF.Sqrt)
                    nc.vector.reciprocal(rkn[:, cols], knrm[:, cols])
                    nc.vector.tensor_mul(cosb[:, cols], dsrc, rkn[:, cols])

            def emit_ctx(g0, g1, pair, stop_last=False):
                p0, p1, first = pair
                for t in range(g0, g1):
                    kt = ktile(t)
                    st = first["v"]
                    first["v"] = False
                    stop = stop_last and t == g1 - 1
                    nc.tensor.matmul(p0[:], cosb[:, t : t + 1],
                                     kt[:, 0:512], start=st, stop=stop)
                    nc.tensor.matmul(p1[:], cosb[:, t : t + 1],
                                     kt[:, 512:1024], start=st, stop=stop)

            pairA = (psA0, psA1, firstA)
            pairB = (psB0, psB1, firstB)

            def emit_hi_window(w):
                # dots for hi rows 512w..512w+511 on the PE; drain to
                # dhfl (DVE/ACT alternating, scheduler-placed), reshape
                # into dh columns 4w..4w+3 via idle-DMA scatter
                for b in range(NB):
                    nc.tensor.matmul(
                        psD[:], qt[:, b : b + 1], kthi_rhs(w, b),
                        start=(b == 0), stop=(b == NB - 1),
                    )
                src = dhfl[:, W_ROWS * w : W_ROWS * (w + 1)]
                if w % 2 == 0:
                    nc.vector.tensor_copy(src, psD[:])
                else:
                    nc.scalar.copy(src, psD[:])
                # kthi column order within a window is j = 4*p + tl, so
                # src is already partition-major: plain [1,512]->[128,4]
                nc.sync.dma_start(dh[:, 4 * w : 4 * (w + 1)], src)

            def emit_lo_group(gi):
                g0, g1 = LO_CHUNKS[gi]
                emit_lo_elem(gi, g0, g1)
                nsrc = (nrm2s if gi in DVE_SQ_GROUPS else nrm2p)
                emit_cos(slice(g0, g1), dots[:, g0:g1], nsrc[:, g0:g1])
                emit_ctx(g0, g1, pairA)

            emit_lo_group(0)
            emit_hi_window(0)
            emit_lo_group(1)
            emit_hi_window(1)
            emit_lo_group(2)
            emit_hi_window(2)
            emit_lo_group(3)
            emit_hi_window(3)
            emit_lo_group(4)

            # hi groups: squares on ACT from the row layout, cos from
            # (dh, nrm2p), ctx from the row layout
            for hg, (g0, g1) in enumerate(HI_CHUNKS):
                for t in range(g0, g1):
                    nc.scalar.activation(
                        actscr[:], ktile(t), AF.Square,
                        accum_out=nrm2p[:, t : t + 1],
                    )
                emit_cos(slice(g0, g1), dh[:, g0 - T_LO : g1 - T_LO],
                         nrm2p[:, g0:g1])
                last = hg == len(HI_CHUNKS) - 1
                if not last:
                    emit_ctx(g0, g1, pairA,
                             stop_last=(hg == len(HI_CHUNKS) - 2))
                else:
                    ctxA = pool.tile([1, H], f32, name="ctxA")
                    nc.scalar.copy(ctxA[:, 0:512], psA0[:])
                    nc.vector.tensor_copy(ctxA[:, 512:1024], psA1[:])
                    nc.sync.dma_start(ctx_d[0:1, :], ctxA[:])
                    emit_ctx(g0, g1, pairB, stop_last=True)

            ctxB = pool.tile([1, H], f32, name="ctxB")
            nc.scalar.copy(ctxB[:, 0:512], psB0[:])
            nc.vector.tensor_copy(ctxB[:, 512:1024], psB1[:])
            nc.sync.dma_start(ctx_d[1:2, :], ctxB[:])

    nc.compile()
    return nc


def _get_nc():
    if "nc" not in _NC_CACHE:
        _NC_CACHE["nc"] = _build_nc()
    return _NC_CACHE["nc"]


def prepare_in_maps(query: np.ndarray, keys: np.ndarray) -> list[dict]:
    query = np.asarray(query, dtype=np.float32)
    keys = np.asarray(keys, dtype=np.float32)
    assert query.shape == (1, H) and keys.shape == (S_FULL, H)

    q = query.reshape(H).astype(np.float64)
    qn = (q / np.linalg.norm(q)).astype(np.float16)
    qb = np.ascontiguousarray(np.broadcast_to(qn[None, :], (P, H)))
    qt = np.ascontiguousarray(qn.reshape(NB, P).T)  # qt[p, b] = qn[128b+p]

    keys16 = keys.astype(np.float16)
    shards = keys16.reshape(N_CORES, S, H)
    in_maps = []
    for i in range(N_CORES):
        sh = shards[i]
        # row layout, t-major: packed[p, t, c] = sh[128t + p, c]
        kp = np.ascontiguousarray(
            sh.reshape(T, P, H).transpose(1, 0, 2)).reshape(P, T * H)
        hi = sh[T_LO * P :]                      # [R_HI, H]
        # kthi[p, w, b, 4*prow + tl] = hi[512w + 128*tl + prow, 128b + p]
        # (within-window columns permuted so the PE's [1,512] dot row is
        # partition-major and reshapes to dh[p, 4w+tl] contiguously)
        kthi = np.ascontiguousarray(
            hi.reshape(N_WIN, 4, P, NB, P).transpose(4, 0, 3, 2, 1)
        ).reshape(P, NB * R_HI)
        in_maps.append({"keys": kp, "kthi": kthi, "qb": qb, "qt": qt})
    return in_maps


def combine_results(results: list[dict]) -> np.ndarray:
    partials = np.stack([results[i]["ctx"] for i in range(N_CORES)])
    out = partials.astype(np.float64).sum(axis=(0, 1)).astype(np.float32)
    return out[None, :]


def kernel(query: np.ndarray, keys: np.ndarray) -> np.ndarray:
    from concourse.bass_utils import run_bass_kernel_spmd

    in_maps = prepare_in_maps(query, keys)
    nc = _get_nc()
    res = run_bass_kernel_spmd(nc, in_maps, list(range(N_CORES)))
    return combine_results(res.results)


# revision 18
# speedup vs baseline: 1.1053x; 1.0868x over previous
"""Bahdanau-style cosine attention kernel for Trainium2 (8 NeuronCores).

reference math (fp32):
    q = squeeze(query)              # [H]
    dots = keys @ q                 # [S]
    cos = dots / (|q| * |keys_i|)   # [S]
    context = sum_i cos_i * keys_i  # [H]

Sharding: keys split along S across 8 cores (4096 rows each); host
normalizes q by |q| and casts everything to fp16 (rel err ~2e-4 vs the
2e-2 gate) so each core streams 8 MiB of keys instead of 16 MiB.

Engine model (measured): free-dim accumulate passes exist only on DVE
(scalar_tensor_tensor ~1.3 us/tile, no 2x uop) and ACT (Square+accum
~1.1 us/tile); 64 passes on two engines is the wall. So rows 2048-4095
are uploaded twice: row layout (squares + context) AND transposed
(kthi), letting the idle PE compute their 16 dot passes as matmuls
with the query as stationary weights: per 512-row window, 8 matmuls of
N=512 accumulate the 8 column blocks into a [1,512] PSUM row; DVE/ACT
drain it to SBUF and an idle-DMA scatter reshapes it into [128, 4]
columns for the cos chain. DVE keeps 16 low dot passes + 6 squares,
ACT 26 squares; both land ~34 us next to a ~31 us DMA stream.

Both tensors are host-packed so every DMA chunk is per-partition
contiguous (128 large descriptors per chunk): HWDGE descriptor
generation on the sync queue is the issue-path bottleneck otherwise.

Context matmuls accumulate in two PSUM bank pairs: pair A (all but the
last hi group) stops and ships its half of the output early; pair B
covers the tail group so the final drain is short.
"""

import os
import sys

import numpy as np

for _p in ("/opt/trn_rl_repo",):
    if os.path.isdir(_p) and _p not in sys.path:
        sys.path.append(_p)

P = 128          # SBUF partitions
H = 1024         # feature dim
NB = H // P      # column blocks = 8
S_FULL = 32768   # full sequence
N_CORES = 8
S = S_FULL // N_CORES   # rows per core = 4096
T = S // P              # row-tiles per core = 32
T_LO = 16               # tiles whose dots run on DVE (rows 0..2047)
T_HI = T - T_LO         # tiles whose dots run on the PE via kthi
R_HI = T_HI * P         # 2048 transposed rows
W_ROWS = 512            # kthi window rows (= 1 PSUM bank of f32)
N_WIN = R_HI // W_ROWS  # 4 windows, 1:1 with the hi groups

# klo chunks (tiles 0..15) double as cos groups
LO_CHUNKS = [(0, 2), (2, 4), (4, 8), (8, 12), (12, 16)]
# khi row-layout chunks (tiles 16..31) = hi groups = kthi windows
HI_CHUNKS = [(16, 20), (20, 24), (24, 28), (28, 32)]
# lo groups whose square runs on DVE (accum in SBUF)
DVE_SQ_GROUPS = frozenset({1, 3})   # tiles 2,3 and 8..11
PE_WARMUP_MMS = 6

_NC_CACHE = {}


def _build_nc():
    import concourse.bacc as bacc
    import concourse.tile as tile
    from concourse import mybir

    f32 = mybir.dt.float32
    f16 = mybir.dt.float16
    AF = mybir.ActivationFunctionType
    OP = mybir.AluOpType
    nc = bacc.Bacc("TRN2", target_bir_lowering=False, debug=False)

    # keys packed [p, t*H + c] = keys[128t + p, c]; kthi packed
    # [p, w*NB*W_ROWS + b*W_ROWS + r] = keys[2048 + 512w + r, 128b + p]
    keys_d = nc.dram_tensor("keys", [P, T * H], f16, kind="ExternalInput").ap()
    kthi_d = nc.dram_tensor("kthi", [P, NB * R_HI], f16,
                            kind="ExternalInput").ap()
    qb_d = nc.dram_tensor("qb", [P, H], f16, kind="ExternalInput").ap()
    qt_d = nc.dram_tensor("qt", [P, NB], f16, kind="ExternalInput").ap()
    ctx_d = nc.dram_tensor("ctx", [2, H], f32, kind="ExternalOutput").ap()

    with tile.TileContext(nc) as tc:
        with (
            tc.tile_pool(name="main", bufs=1) as pool,
            tc.tile_pool(name="psum", bufs=1, space="PSUM") as pp,
        ):
            qb = pool.tile([P, H], f16, name="qb_sb")
            qt = pool.tile([P, NB], f16, name="qt_sb")
            nc.sync.dma_start(qb[:], qb_d[:])
            nc.sync.dma_start(qt[:], qt_d[:])

            kthi = pool.tile([P, NB * R_HI], f16, name="kthi_sb")
            kcs = {}

            def load_keys(t0, t1, name):
                kc = pool.tile([P, (t1 - t0) * H], f16, name=name, tag=name)
                nc.sync.dma_start(kc[:], keys_d[:, t0 * H : t1 * H])
                for i in range(t0, t1):
                    kcs[i] = (kc, i - t0)

            def load_kthi(w):
                wb = NB * W_ROWS
                nc.sync.dma_start(kthi[:, w * wb : (w + 1) * wb],
                                  kthi_d[:, w * wb : (w + 1) * wb])

            def ktile(t):
                kc, i = kcs[t]
                return kc[:, i * H : (i + 1) * H]

            def kthi_rhs(w, b):
                base = w * NB * W_ROWS + b * W_ROWS
                return kthi[:, base : base + W_ROWS]

            # DMA issue order: lo, transposed and hi row chunks woven so
            # every consumer engine gets fed continuously and no group's
            # entire pipeline is parked at the stream end
            load_keys(0, 2, "klo0")
            load_keys(2, 4, "klo1")
            load_kthi(0)
            load_keys(16, 20, "khi0")
            load_keys(4, 8, "klo2")
            load_kthi(1)
            load_keys(20, 24, "khi1")
            load_keys(8, 12, "klo3")
            load_kthi(2)
            load_keys(24, 28, "khi2")
            load_keys(12, 16, "klo4")
            load_kthi(3)
            load_keys(28, 32, "khi3")

            dots = pool.tile([P, T_LO], f32, name="dots")
            dh = pool.tile([P, T_HI], f32, name="dh")
            dhfl = pool.tile([1, R_HI], f32, name="dhfl")
            nrm2p = pp.tile([P, T], f32, name="nrm2p")
            nrm2s = pool.tile([P, T_LO], f32, name="nrm2s")
            knrm = pool.tile([P, T], f32, name="knrm")
            rkn = pool.tile([P, T], f32, name="rkn")
            cosb = pool.tile([P, T], f16, name="cosb")
            dvescr = pool.tile([P, H], f16, name="dvescr")
            actscr = pp.tile([P, H], f32, name="actscr")
            psD = pp.tile([1, W_ROWS], f32, name="psD")
            psA0 = pp.tile([1, 512], f32, name="psA0")
            psA1 = pp.tile([1, 512], f32, name="psA1")
            psB0 = pp.tile([1, 512], f32, name="psB0")
            psB1 = pp.tile([1, 512], f32, name="psB1")

            for _ in range(PE_WARMUP_MMS):
                nc.tensor.matmul(psB0[:], qb[:, 0:1], qb[:, 0:512],
                                 start=True, stop=True)

            firstA = {"v": True}
            firstB = {"v": True}

            def emit_lo_elem(gi, g0, g1):
                dve_sq = gi in DVE_SQ_GROUPS
                for t in range(g0, g1):
                    nc.vector.scalar_tensor_tensor(
                        out=dvescr[:], in0=ktile(t), scalar=1.0, in1=qb[:],
                        op0=OP.mult, op1=OP.mult,
                        accum_out=dots[:, t : t + 1],
                    )
                    if dve_sq:
                        nc.vector.scalar_tensor_tensor(
                            out=dvescr[:], in0=ktile(t), scalar=1.0,
                            in1=ktile(t), op0=OP.mult, op1=OP.mult,
                            accum_out=nrm2s[:, t : t + 1],
                        )
                    else:
                        nc.scalar.activation(
                            actscr[:], ktile(t), AF.Square,
                            accum_out=nrm2p[:, t : t + 1],
                        )

            def emit_cos(cols, dsrc, nsrc):
                with tc.high_priority(offset=40):
                    nc.scalar.activation(knrm[:, cols], nsrc, AF.Sqrt)
                    nc.vector.reciprocal(rkn[:, cols], knrm[:, cols])
                    nc.vector.tensor_mul(cosb[:, cols], dsrc, rkn[:, cols])

            def emit_ctx(g0, g1, pair, stop_last=False):
                p0, p1, first = pair
                for t in range(g0, g1):
                    kt = ktile(t)
                    st = first["v"]
                    first["v"] = False
                    stop = stop_last and t == g1 - 1
                    nc.tensor.matmul(p0[:], cosb[:, t : t + 1],
                                     kt[:, 0:512], start=st, stop=stop)
                    nc.tensor.matmul(p1[:], cosb[:, t : t + 1],
                                     kt[:, 512:1024], start=st, stop=stop)

            pairA = (psA0, psA1, firstA)
            pairB = (psB0, psB1, firstB)

            def emit_hi_mms(w):
                # dots for hi rows 512w..512w+511 on the PE
                for b in range(NB):
                    nc.tensor.matmul(
                        psD[:], qt[:, b : b + 1], kthi_rhs(w, b),
                        start=(b == 0), stop=(b == NB - 1),
                    )

            def emit_hi_drain(w, eng):
                # drain psD to dhfl, then idle-DMA scatter into dh
                # columns 4w..4w+3 (kthi column order within a window is
                # j = 4*p + tl, so the [1,512] row is partition-major)
                src = dhfl[:, W_ROWS * w : W_ROWS * (w + 1)]
                if eng == "dve":
                    nc.vector.tensor_copy(src, psD[:])
                else:
                    nc.scalar.copy(src, psD[:])
                nc.gpsimd.dma_start(dh[:, 4 * w : 4 * (w + 1)], src)

            def emit_hi_sq(hg):
                g0, g1 = HI_CHUNKS[hg]
                for t in range(g0, g1):
                    nc.scalar.activation(
                        actscr[:], ktile(t), AF.Square,
                        accum_out=nrm2p[:, t : t + 1],
                    )

            def emit_hi_cos_ctx(hg, pair, stop_last=False):
                g0, g1 = HI_CHUNKS[hg]
                emit_cos(slice(g0, g1), dh[:, g0 - T_LO : g1 - T_LO],
                         nrm2p[:, g0:g1])
                emit_ctx(g0, g1, pair, stop_last=stop_last)

            def emit_lo_group(gi):
                g0, g1 = LO_CHUNKS[gi]
                emit_lo_elem(gi, g0, g1)
                nsrc = (nrm2s if gi in DVE_SQ_GROUPS else nrm2p)
                emit_cos(slice(g0, g1), dots[:, g0:g1], nsrc[:, g0:g1])
                emit_ctx(g0, g1, pairA)

            emit_lo_group(0)
            emit_hi_mms(0)
            emit_hi_drain(0, "dve")
            emit_hi_sq(0)
            emit_lo_group(1)
            emit_hi_cos_ctx(0, pairA)
            emit_hi_mms(1)
            emit_hi_drain(1, "act")
            emit_lo_group(2)
            emit_hi_sq(1)
            emit_hi_mms(2)
            emit_hi_drain(2, "dve")
            emit_hi_cos_ctx(1, pairA)
            emit_lo_group(3)
            emit_hi_sq(2)
            emit_hi_mms(3)
            emit_lo_group(4)
            emit_hi_cos_ctx(2, pairA, stop_last=True)
            emit_hi_drain(3, "dve")
            emit_hi_sq(3)
            # pair A drains and ships while the tail group finishes
            ctxA = pool.tile([1, H], f32, name="ctxA")
            nc.scalar.copy(ctxA[:, 0:512], psA0[:])
            nc.vector.tensor_copy(ctxA[:, 512:1024], psA1[:])
            nc.sync.dma_start(ctx_d[0:1, :], ctxA[:])
            emit_hi_cos_ctx(3, pairB, stop_last=True)

            ctxB = pool.tile([1, H], f32, name="ctxB")
            nc.scalar.copy(ctxB[:, 0:512], psB0[:])
            nc.vector.tensor_copy(ctxB[:, 512:1024], psB1[:])
            nc.sync.dma_start(ctx_d[1:2, :], ctxB[:])

    nc.compile()
    return nc


def _get_nc():
    if "nc" not in _NC_CACHE:
        _NC_CACHE["nc"] = _build_nc()
    return _NC_CACHE["nc"]


def prepare_in_maps(query: np.ndarray, keys: np.ndarray) -> list[dict]:
    query = np.asarray(query, dtype=np.float32)
    keys = np.asarray(keys, dtype=np.float32)
    assert query.shape == (1, H) and keys.shape == (S_FULL, H)

    q = query.reshape(H).astype(np.float64)
    qn = (q / np.linalg.norm(q)).astype(np.float16)
    qb = np.ascontiguousarray(np.broadcast_to(qn[None, :], (P, H)))
    qt = np.ascontiguousarray(qn.reshape(NB, P).T)  # qt[p, b] = qn[128b+p]

    keys16 = keys.astype(np.float16)
    shards = keys16.reshape(N_CORES, S, H)
    in_maps = []
    for i in range(N_CORES):
        sh = shards[i]
        # row layout, t-major: packed[p, t, c] = sh[128t + p, c]
        kp = np.ascontiguousarray(
            sh.reshape(T, P, H).transpose(1, 0, 2)).reshape(P, T * H)
        hi = sh[T_LO * P :]                      # [R_HI, H]
        # kthi[p, w, b, 4*prow + tl] = hi[512w + 128*tl + prow, 128b + p]
        # (within-window columns permuted so the PE's [1,512] dot row is
        # partition-major and reshapes to dh[p, 4w+tl] contiguously)
        kthi = np.ascontiguousarray(
            hi.reshape(N_WIN, 4, P, NB, P).transpose(4, 0, 3, 2, 1)
        ).reshape(P, NB * R_HI)
        in_maps.append({"keys": kp, "kthi": kthi, "qb": qb, "qt": qt})
    return in_maps


def combine_results(results: list[dict]) -> np.ndarray:
    partials = np.stack([results[i]["ctx"] for i in range(N_CORES)])
    out = partials.astype(np.float64).sum(axis=(0, 1)).astype(np.float32)
    return out[None, :]


def kernel(query: np.ndarray, keys: np.ndarray) -> np.ndarray:
    from concourse.bass_utils import run_bass_kernel_spmd

    in_maps = prepare_in_maps(query, keys)
    nc = _get_nc()
    res = run_bass_kernel_spmd(nc, in_maps, list(range(N_CORES)))
    return combine_results(res.results)


# revision 21
# speedup vs baseline: 1.1743x; 1.0624x over previous
"""Bahdanau-style cosine attention kernel for Trainium2 (8 NeuronCores).

reference math (fp32):
    q = squeeze(query)              # [H]
    dots = keys @ q                 # [S]
    cos = dots / (|q| * |keys_i|)   # [S]
    context = sum_i cos_i * keys_i  # [H]

Sharding: keys split along S across 8 cores (4096 rows each); host
normalizes q by |q| and casts everything to fp16 (rel err ~2e-4 vs the
2e-2 gate) so each core streams 8 MiB of keys instead of 16 MiB.

Engine model (measured): free-dim accumulate passes exist only on DVE
(scalar_tensor_tensor ~1.3 us/tile, no 2x uop) and ACT (Square+accum
~1.1 us/tile); 64 passes on two engines is the wall. So rows 2048-4095
are uploaded twice: row layout (squares + context) AND transposed
(kthi), letting the idle PE compute their 16 dot passes as matmuls
with the query as stationary weights: per 512-row window, 8 matmuls of
N=512 accumulate the 8 column blocks into a [1,512] PSUM row; DVE/ACT
drain it to SBUF and an idle-DMA scatter reshapes it into [128, 4]
columns for the cos chain. DVE keeps 16 low dot passes + 6 squares,
ACT 26 squares; both land ~34 us next to a ~31 us DMA stream.

Both tensors are host-packed so every DMA chunk is per-partition
contiguous (128 large descriptors per chunk): HWDGE descriptor
generation on the sync queue is the issue-path bottleneck otherwise.

Context matmuls accumulate in two PSUM bank pairs: pair A (all but the
last hi group) stops and ships its half of the output early; pair B
covers the tail group so the final drain is short.
"""

import os
import sys

import numpy as np

for _p in ("/opt/trn_rl_repo",):
    if os.path.isdir(_p) and _p not in sys.path:
        sys.path.append(_p)

P = 128          # SBUF partitions
H = 1024         # feature dim
NB = H // P      # column blocks = 8
S_FULL = 32768   # full sequence
N_CORES = 8
S = S_FULL // N_CORES   # rows per core = 4096
T = S // P              # row-tiles per core = 32
T_LO = 16               # tiles whose dots run on DVE (rows 0..2047)
T_HI = T - T_LO         # tiles whose dots run on the PE via kthi
R_HI = T_HI * P         # 2048 transposed rows
W_ROWS = 512            # kthi window rows (= 1 PSUM bank of f32)
N_WIN = R_HI // W_ROWS  # 4 windows, 1:1 with the hi groups

# klo chunks (tiles 0..15) double as cos groups
LO_CHUNKS = [(0, 2), (2, 4), (4, 8), (8, 12), (12, 16)]
# khi row-layout chunks (tiles 16..31) = hi groups = kthi windows
HI_CHUNKS = [(16, 20), (20, 24), (24, 28), (28, 32)]
# lo groups whose square runs on DVE (accum in SBUF)
DVE_SQ_GROUPS = frozenset({1, 3, 4})   # tiles 2,3 and 8..15
PE_WARMUP_MMS = 6

_NC_CACHE = {}


def _build_nc():
    import concourse.bacc as bacc
    import concourse.tile as tile
    from concourse import mybir

    f32 = mybir.dt.float32
    f16 = mybir.dt.float16
    AF = mybir.ActivationFunctionType
    OP = mybir.AluOpType
    nc = bacc.Bacc("TRN2", target_bir_lowering=False, debug=False)

    # keys packed [p, t*H + c] = keys[128t + p, c]; kthi packed
    # [p, w*NB*W_ROWS + b*W_ROWS + r] = keys[2048 + 512w + r, 128b + p]
    keys_d = nc.dram_tensor("keys", [P, T * H], f16, kind="ExternalInput").ap()
    kthi_d = nc.dram_tensor("kthi", [P, NB * R_HI], f16,
                            kind="ExternalInput").ap()
    qb_d = nc.dram_tensor("qb", [P, H], f16, kind="ExternalInput").ap()
    qt_d = nc.dram_tensor("qt", [P, NB], f16, kind="ExternalInput").ap()
    ctx_d = nc.dram_tensor("ctx", [2, H], f32, kind="ExternalOutput").ap()

    with tile.TileContext(nc) as tc:
        with (
            tc.tile_pool(name="main", bufs=1) as pool,
            tc.tile_pool(name="psum", bufs=1, space="PSUM") as pp,
        ):
            qb = pool.tile([P, H], f16, name="qb_sb")
            qt = pool.tile([P, NB], f16, name="qt_sb")
            nc.sync.dma_start(qb[:], qb_d[:])
            nc.sync.dma_start(qt[:], qt_d[:])

            kthi = pool.tile([P, NB * R_HI], f16, name="kthi_sb")
            kcs = {}

            def load_keys(t0, t1, name):
                kc = pool.tile([P, (t1 - t0) * H], f16, name=name, tag=name)
                nc.sync.dma_start(kc[:], keys_d[:, t0 * H : t1 * H])
                for i in range(t0, t1):
                    kcs[i] = (kc, i - t0)

            def load_kthi(w):
                wb = NB * W_ROWS
                nc.sync.dma_start(kthi[:, w * wb : (w + 1) * wb],
                                  kthi_d[:, w * wb : (w + 1) * wb])

            def ktile(t):
                kc, i = kcs[t]
                return kc[:, i * H : (i + 1) * H]

            def kthi_rhs(w, b):
                base = w * NB * W_ROWS + b * W_ROWS
                return kthi[:, base : base + W_ROWS]

            # DMA issue order: lo, transposed and hi row chunks woven so
            # every consumer engine gets fed continuously and no group's
            # entire pipeline is parked at the stream end
            load_keys(0, 2, "klo0")
            load_keys(2, 4, "klo1")
            load_kthi(0)
            load_keys(16, 20, "khi0")
            load_keys(4, 8, "klo2")
            load_kthi(1)
            load_keys(20, 24, "khi1")
            load_keys(8, 12, "klo3")
            load_keys(12, 16, "klo4")
            load_kthi(2)
            load_keys(24, 28, "khi2")
            load_kthi(3)
            load_keys(28, 32, "khi3")

            dots = pool.tile([P, T_LO], f32, name="dots")
            dh = pool.tile([P, T_HI], f32, name="dh")
            dhfl = pool.tile([1, R_HI], f32, name="dhfl")
            nrm2p = pp.tile([P, T], f32, name="nrm2p")
            nrm2s = pool.tile([P, T_LO], f32, name="nrm2s")
            knrm = pool.tile([P, T], f32, name="knrm")
            rkn = pool.tile([P, T], f32, name="rkn")
            cosb = pool.tile([P, T], f16, name="cosb")
            dvescr = pool.tile([P, H], f16, name="dvescr")
            actscr = pp.tile([P, H], f32, name="actscr")
            psD = pp.tile([1, W_ROWS], f32, name="psD")
            psA0 = pp.tile([1, 512], f32, name="psA0")
            psA1 = pp.tile([1, 512], f32, name="psA1")
            psB0 = pp.tile([1, 512], f32, name="psB0")
            psB1 = pp.tile([1, 512], f32, name="psB1")

            for _ in range(PE_WARMUP_MMS):
                nc.tensor.matmul(psB0[:], qb[:, 0:1], qb[:, 0:512],
                                 start=True, stop=True)

            firstA = {"v": True}
            firstB = {"v": True}

            def emit_lo_elem(gi, g0, g1):
                dve_sq = gi in DVE_SQ_GROUPS
                for t in range(g0, g1):
                    nc.vector.scalar_tensor_tensor(
                        out=dvescr[:], in0=ktile(t), scalar=1.0, in1=qb[:],
                        op0=OP.mult, op1=OP.mult,
                        accum_out=dots[:, t : t + 1],
                    )
                    if dve_sq:
                        nc.vector.scalar_tensor_tensor(
                            out=dvescr[:], in0=ktile(t), scalar=1.0,
                            in1=ktile(t), op0=OP.mult, op1=OP.mult,
                            accum_out=nrm2s[:, t : t + 1],
                        )
                    else:
                        nc.scalar.activation(
                            actscr[:], ktile(t), AF.Square,
                            accum_out=nrm2p[:, t : t + 1],
                        )

            def emit_cos(cols, dsrc, nsrc):
                with tc.high_priority(offset=40):
                    nc.scalar.activation(knrm[:, cols], nsrc, AF.Sqrt)
                    nc.vector.reciprocal(rkn[:, cols], knrm[:, cols])
                    nc.vector.tensor_mul(cosb[:, cols], dsrc, rkn[:, cols])

            def emit_ctx(g0, g1, pair, stop_last=False):
                p0, p1, first = pair
                for t in range(g0, g1):
                    kt = ktile(t)
                    st = first["v"]
                    first["v"] = False
                    stop = stop_last and t == g1 - 1
                    nc.tensor.matmul(p0[:], cosb[:, t : t + 1],
                                     kt[:, 0:512], start=st, stop=stop)
                    nc.tensor.matmul(p1[:], cosb[:, t : t + 1],
                                     kt[:, 512:1024], start=st, stop=stop)

            pairA = (psA0, psA1, firstA)
            pairB = (psB0, psB1, firstB)

            def emit_hi_mms(w):
                # dots for hi rows 512w..512w+511 on the PE
                for b in range(NB):
                    nc.tensor.matmul(
                        psD[:], qt[:, b : b + 1], kthi_rhs(w, b),
                        start=(b == 0), stop=(b == NB - 1),
                    )

            def emit_hi_drain(w, eng):
                # drain psD to dhfl, then idle-DMA scatter into dh
                # columns 4w..4w+3 (kthi column order within a window is
                # j = 4*p + tl, so the [1,512] row is partition-major)
                src = dhfl[:, W_ROWS * w : W_ROWS * (w + 1)]
                if eng == "dve":
                    nc.vector.tensor_copy(src, psD[:])
                else:
                    nc.scalar.copy(src, psD[:])
                nc.gpsimd.dma_start(dh[:, 4 * w : 4 * (w + 1)], src)

            def emit_hi_sq(hg):
                g0, g1 = HI_CHUNKS[hg]
                for t in range(g0, g1):
                    nc.scalar.activation(
                        actscr[:], ktile(t), AF.Square,
                        accum_out=nrm2p[:, t : t + 1],
                    )

            def emit_hi_cos_ctx(hg, pair, stop_last=False):
                g0, g1 = HI_CHUNKS[hg]
                emit_cos(slice(g0, g1), dh[:, g0 - T_LO : g1 - T_LO],
                         nrm2p[:, g0:g1])
                emit_ctx(g0, g1, pair, stop_last=stop_last)

            def emit_lo_group(gi):
                g0, g1 = LO_CHUNKS[gi]
                emit_lo_elem(gi, g0, g1)
                nsrc = (nrm2s if gi in DVE_SQ_GROUPS else nrm2p)
                emit_cos(slice(g0, g1), dots[:, g0:g1], nsrc[:, g0:g1])
                emit_ctx(g0, g1, pairA)

            # g4 (DVE-squared, arrives mid-stream) is the pair-B tail:
            # its chain runs right after the DVE queue drains while the
            # hi groups (ACT-gated) finish inside pair A and ship early
            emit_lo_group(0)
            emit_hi_mms(0)
            emit_hi_drain(0, "dve")
            emit_hi_sq(0)
            emit_lo_group(1)
            emit_hi_cos_ctx(0, pairA)
            emit_hi_mms(1)
            emit_hi_drain(1, "act")
            emit_lo_group(2)
            emit_hi_sq(1)
            emit_hi_mms(2)
            emit_hi_drain(2, "act")
            emit_hi_cos_ctx(1, pairA)
            emit_lo_group(3)
            emit_hi_sq(2)
            emit_hi_mms(3)
            g0, g1 = LO_CHUNKS[4]
            emit_lo_elem(4, g0, g1)
            emit_hi_cos_ctx(2, pairA)
            emit_hi_drain(3, "act")
            emit_hi_sq(3)
            emit_hi_cos_ctx(3, pairA, stop_last=True)
            # pair A drains and ships while the g4 tail finishes on DVE
            ctxA = pool.tile([1, H], f32, name="ctxA")
            nc.scalar.copy(ctxA[:, 0:512], psA0[:])
            nc.vector.tensor_copy(ctxA[:, 512:1024], psA1[:])
            nc.sync.dma_start(ctx_d[0:1, :], ctxA[:])
            emit_cos(slice(g0, g1), dots[:, g0:g1], nrm2s[:, g0:g1])
            emit_ctx(g0, g1, pairB, stop_last=True)

            ctxB = pool.tile([1, H], f32, name="ctxB")
            nc.scalar.copy(ctxB[:, 0:512], psB0[:])
            nc.vector.tensor_copy(ctxB[:, 512:1024], psB1[:])
            nc.sync.dma_start(ctx_d[1:2, :], ctxB[:])

    nc.compile()
    return nc


def _get_nc():
    if "nc" not in _NC_CACHE:
        _NC_CACHE["nc"] = _build_nc()
    return _NC_CACHE["nc"]


def prepare_in_maps(query: np.ndarray, keys: np.ndarray) -> list[dict]:
    query = np.asarray(query, dtype=np.float32)
    keys = np.asarray(keys, dtype=np.float32)
    assert query.shape == (1, H) and keys.shape == (S_FULL, H)

    q = query.reshape(H).astype(np.float64)
    qn = (q / np.linalg.norm(q)).astype(np.float16)
    qb = np.ascontiguousarray(np.broadcast_to(qn[None, :], (P, H)))
    qt = np.ascontiguousarray(qn.reshape(NB, P).T)  # qt[p, b] = qn[128b+p]

    keys16 = keys.astype(np.float16)
    shards = keys16.reshape(N_CORES, S, H)
    in_maps = []
    for i in range(N_CORES):
        sh = shards[i]
        # row layout, t-major: packed[p, t, c] = sh[128t + p, c]
        kp = np.ascontiguousarray(
            sh.reshape(T, P, H).transpose(1, 0, 2)).reshape(P, T * H)
        hi = sh[T_LO * P :]                      # [R_HI, H]
        # kthi[p, w, b, 4*prow + tl] = hi[512w + 128*tl + prow, 128b + p]
        # (within-window columns permuted so the PE's [1,512] dot row is
        # partition-major and reshapes to dh[p, 4w+tl] contiguously)
        kthi = np.ascontiguousarray(
            hi.reshape(N_WIN, 4, P, NB, P).transpose(4, 0, 3, 2, 1)
        ).reshape(P, NB * R_HI)
        in_maps.append({"keys": kp, "kthi": kthi, "qb": qb, "qt": qt})
    return in_maps


def combine_results(results: list[dict]) -> np.ndarray:
    partials = np.stack([results[i]["ctx"] for i in range(N_CORES)])
    out = partials.astype(np.float64).sum(axis=(0, 1)).astype(np.float32)
    return out[None, :]


def kernel(query: np.ndarray, keys: np.ndarray) -> np.ndarray:
    from concourse.bass_utils import run_bass_kernel_spmd

    in_maps = prepare_in_maps(query, keys)
    nc = _get_nc()
    res = run_bass_kernel_spmd(nc, in_maps, list(range(N_CORES)))
    return combine_results(res.results)
